# revision 32
# baseline (speedup 1.0000x reference)
"""Trainium2 Bass kernel for nn_Attention_4063039062503.

Reference (per batch b, C=128 channels, N=4096 points):
    q = W1 @ x + b1;  k = W2 @ x + b2          # [C, N]
    s[n, m] = q[:, n] . k[:, m]                # [N, N]
    a = softmax(s, axis=m)
    out = relu(x + x @ a.T)

Math restructure (the projections collapse into one tiny GEMM):
    KtQ = x_K^T (W2^T W1) x_q + u 1^T + 1 v^T + const,  u = x_K^T (W2^T b1)
    The v/const terms are constant over keys -> cancel in softmax.
    So  S_eff^T = x_K^T Z'   with   Z' = A^T... Z' = (W2^T W1) x_q + w 1^T,
    w = W2^T b1.  Host precomputes A = W1^T W2 (lhsT of the Z-proj) and w;
    the device does Z' = matmul(A, x_q) + w (bias folded into the PSUM
    evacuation) and never touches W1/W2/b1/b2 again.  exp() then needs only
    a constant -30 shift -> the ACT engine does nothing but 64 pure exps.

Sharding: 8 cores, core i -> batch i//2, query half i%2 (2048 queries),
full 4096 keys local (no collectives).  Keys are ROTATED per core so the
query half is always columns 0:2048 -> one ascending DMA stream feeds the
Z-projection and the early S-tiles.

Per-core pipeline (flash-attention style, flattened 64-iteration stream):
  - S^T tile [m=128, 1024 q] per (pass, m-tile) on TensorE in fp16
  - exp(s - 30) on ScalarE PSUM->SBUF, bf16 out; ACT runs back-to-back
    (steady-state cadence ~1.01-1.21us/m-tile, ACT/PE co-bound)
  - O[c, q] += xT[m-tile] @ E on TensorE bf16, fp32 PSUM accumulation,
    lagged DLAG=3 m-tiles behind the exp stream (DLAG=2 couples PE<->ACT
    through semaphore latency and costs ~15%)
  - row-sums: DVE bf16 accumulate + 3-stage ones-matmul (stages 2-3 read
    the last two E-tiles directly so the reciprocal starts immediately)
  - pass-0 uses ps_o for O and ps_r for rowsum; pass-1 SWAPS them (O in
    the 2-bank [C,1024] r-slot, rowsums in the o-slots) so pass-1's first
    O-matmul never waits on pass-0's tail reads
  - tail: reciprocal_approx_fast + normalize + residual on DVE; final
    relu on ACT (free after the last exp), DMA out
  - startup: warmups interleaved with the Z-projection halves; xk/xt are
    split into early/bulk SBUF tiles because DMA-completion waits are
    whole-tile and per-queue (a reader of any chunk waits for the queue's
    full counter); bulk transfers must stay on the SP queue (the gpsimd /
    ACT DMA paths are ~2x slower for the same bytes)
"""
from contextlib import ExitStack

import numpy as np
import ml_dtypes

import concourse.tile as tile
from concourse import bacc, mybir
from concourse.bass_utils import run_bass_kernel_spmd

B = 4
C = 128
N = 4096            # keys per batch
NQ = 2048           # queries per core
PW = 1024           # queries per pass
MT = 128            # m (key) tile
N_MT = N // MT      # 32
N_P = NQ // PW      # 2 passes
TOT = N_P * N_MT    # 64 global iterations
DLAG = 3            # O-matmul lag (in m-tiles) behind the S/exp stream
MMF = 512           # max matmul free size (1024 is rejected by the ISA:
                    # a matmul's PSUM output cannot cross a 2KB bank)

F32 = mybir.dt.float32
F16 = mybir.dt.float16
BF16 = mybir.dt.bfloat16
AF = mybir.ActivationFunctionType
ALU = mybir.AluOpType


def build_nc():
    nc = bacc.Bacc("TRN2", target_bir_lowering=False, debug=False, num_devices=8)
    # cols 0..C-1 = A = W1^T W2; col C = w = W2^T b1 (fp16 is plenty for w)
    # cols 0..C-1 = W1^T W2 (Z-proj lhsT); col C = w; cols C+2..2C+1 =
    # W2^T W1 (lhsT of the m-tile-0 fast path G0 = (W2^T W1) xk_tile0)
    a_ext = nc.declare_dram_parameter("a16", [C, 2 * C + 2], F16, isOutput=False)
    xk_ext = nc.declare_dram_parameter("xk", [C, N], F16, isOutput=False)
    xt_ext = nc.declare_dram_parameter("xt", [C, N], BF16, isOutput=False)
    out_ext = nc.declare_dram_parameter("out", [C, NQ], F16, isOutput=True)

    def mm(out_ap, lhsT, rhs, start=True, stop=True):
        wtot = out_ap.shape[-1]
        o = 0
        while o < wtot:
            wd = min(MMF, wtot - o)
            nc.tensor.matmul(out_ap[:, o:o + wd], lhsT, rhs[:, o:o + wd],
                             start=start, stop=stop)
            o += wd

    with ExitStack() as ctx:
        tc = ctx.enter_context(tile.TileContext(nc))
        sb1 = ctx.enter_context(tc.tile_pool(name="sb1", bufs=1))
        consts = sb_in = sb_z = sb_e = sb1
        sb2 = ctx.enter_context(tc.tile_pool(name="sb2", bufs=2))
        sb_acc = sb_tail = sb2
        ps_s = ctx.enter_context(tc.tile_pool(name="ps_s", bufs=2, space="PSUM"))
        ps_o = ctx.enter_context(tc.tile_pool(name="ps_o", bufs=2, space="PSUM"))
        ps_r = ctx.enter_context(tc.tile_pool(name="ps_r", bufs=1, space="PSUM"))

        # warm the PE's HAM clock gate (~3.4us of matmul activity) during
        # the input-DMA wait so the main stream runs at 2.4 GHz; its
        # memset goes first so the warmup starts before the first DMA lands
        # memset on GpSimd: its queue starts ~1us earlier than DVE's, so
        # the warmup (and with it the whole PE stream) begins sooner
        # ALL warmups go before the first data-dependent matmul (y0): PE
        # executes in program order, so a warmup placed after a data-gated
        # matmul lands inside the critical prefix instead of the DMA wait
        wmm = consts.tile([C, 512], BF16, tag="wmm")
        nc.gpsimd.memset(wmm[:], 0.0)
        # ONE psum tile shared by every warmup (WAW-serialized), so the
        # warmups consume a single ps_s pool slot and the pool rotation
        # for the real S tiles stays clean
        wps = ps_s.tile([C, PW], F32, tag="s", name="wps")
        def warm_mm(k):
            for _ in range(k):
                nc.tensor.matmul(wps[:, 0:512], wmm[:, 0:C], wmm[:],
                                 start=True, stop=True)
        warm_mm(4)
        ones_bf = consts.tile([C, C], BF16, tag="ones_bf")
        nc.vector.memset(ones_bf[:], 1.0)
        shift = consts.tile([C, 1], F32, tag="shift")
        nc.vector.memset(shift[:], -30.0)
        zero0 = consts.tile([C, 1], F32, tag="zero0")
        nc.vector.memset(zero0[:], 0.0)
        # warm the exp table early (ACT_TABLE_LOAD ~2.7us)
        warm = consts.tile([1, 16], F32, tag="warm")
        nc.vector.memset(warm[:], 0.0)
        warm_o = consts.tile([1, 16], F32, tag="warm_o")
        nc.scalar.activation(warm_o[:], warm[:], AF.Exp, bias=zero0[0:1, 0:1])

        a16 = sb_in.tile([C, 2 * C + 2], F16, tag="a16")
        # xk/xt split into an early tile (first 8 m-tiles + queries) and a
        # bulk tile: dependency tracking is whole-tile, so a single tile
        # would make every reader wait for the LAST of its DMA chunks
        # xk_a further split in two: Z0a gates on just 0.25MB landing
        # (the transfer time under 8-core HBM contention IS the startup)
        xk_a1 = sb_in.tile([C, 512], F16, tag="xk_a1")
        xk_a2 = sb_in.tile([C, 512], F16, tag="xk_a2")
        xk_b = sb_in.tile([C, N - PW], F16, tag="xk_b")
        xt_a = sb_in.tile([C, PW], BF16, tag="xt_a")
        xt_b = sb_in.tile([C, N - PW], BF16, tag="xt_b")
        # residual x_q recovered from the fp16 xk upload (saves a 1MB DMA;
        # fp16->bf16 residual error ~0.4% of |x| vs the 2e-2*scale gate);
        # bf16 so the tail residual-adds run in the DVE 2x perf mode
        xq16 = sb_in.tile([C, NQ], BF16, tag="xq16")

        def xk_tile(mt):
            if mt < 4:
                return xk_a1[:, mt * MT:(mt + 1) * MT]
            if mt < 8:
                return xk_a2[:, (mt - 4) * MT:(mt - 3) * MT]
            return xk_b[:, (mt - PW // MT) * MT:(mt - PW // MT + 1) * MT]

        def xt_tile(mt):
            if mt < PW // MT:
                return xt_a[:, mt * MT:(mt + 1) * MT]
            return xt_b[:, (mt - PW // MT) * MT:(mt - PW // MT + 1) * MT]
        zt = sb_z.tile([C, NQ], F16, tag="zt")
        e_stage = sb_e.tile([C, N_MT * PW], BF16, tag="e")

        # input DMAs, gating-first order: the first S-matmul half needs
        # only xk[:, 0:512] + a16 + w; all on the SP hwdge queue
        # (ACT-queue gens are slow for big DMAs, gpsimd is SWDGE = slower
        # still); gating ones first so the serial ~600ns descriptor gens
        # start their transfers earliest; xt_a is 4th (first needed by
        # O(0), ~2us after the S stream starts)
        # a16/xk_a1 split so the y0 fast path (W2^T W1 block + first m-tile)
        # gates on the smallest possible transfer under 8-core contention
        nc.sync.dma_start(a16[:, C + 2:2 * C + 2], a_ext[:, C + 2:2 * C + 2])
        nc.sync.dma_start(xk_a1[:, 0:MT], xk_ext[:, 0:MT])
        nc.sync.dma_start(xk_a1[:, MT:512], xk_ext[:, MT:512])
        nc.sync.dma_start(a16[:, 0:C + 2], a_ext[:, 0:C + 2])
        nc.sync.dma_start(xk_a2[:], xk_ext[:, 512:PW])
        nc.sync.dma_start(xt_a[:], xt_ext[:, 0:PW])
        # bulk in four chunks: shorter bursts interleave better with the
        # other seven cores' concurrent input DMAs (measured: the merged
        # 2-way variant pushed all-DMA-complete from ~12us to ~15us)
        nc.sync.dma_start(xk_b[:, 0:PW], xk_ext[:, PW:2 * PW])
        nc.sync.dma_start(xt_b[:, 0:PW], xt_ext[:, PW:2 * PW])
        nc.sync.dma_start(xk_b[:, PW:N - PW], xk_ext[:, 2 * PW:N])
        nc.sync.dma_start(xt_b[:, PW:N - PW], xt_ext[:, 2 * PW:N])
        # fp32 copy of the w column for use as evac bias (DVE scalars and
        # ACT bias want fp32)
        wf32 = consts.tile([C, 1], F32, tag="wf32")
        nc.vector.tensor_copy(wf32[:], a16[:, C:C + 1])

        # ---- prologue under explicit scheduler floors -------------------
        # The tile scheduler orders by ITS sim-readiness (its DMA model
        # knows nothing of 8-core HBM contention), so every prologue op
        # gets a floor that pins the per-engine queue order to the one
        # that matches real data-arrival times.  Floors are sim-only; HW
        # execution remains dependency-driven.
        # m-tile-0 fast path: S(0) = G0^T x_q with G0 = (W2^T W1) xk_tile0
        # (skips the Z-evac on the first-exp critical chain); the u-bias for
        # tile 0 comes from a 1-column matmul into the exp bias.
        # S(0) halves land in SEPARATE psum tiles so exp(0)A gates only on
        # the xk_a1 transfer, not on xk_a2.
        with tc.tile_wait_until(0.0008):
            y0 = ps_o.tile([C, MT], F32, tag="o", name="y0")
            nc.tensor.matmul(y0[:], a16[:, C + 2:2 * C + 2], xk_a1[:, 0:MT],
                             start=True, stop=True)
            y0t = sb_z.tile([C, MT], F16, tag="y0t")
            nc.scalar.activation(y0t[:], y0[:], AF.Identity,
                                 bias=zero0[:, 0:1])
        with tc.tile_wait_until(0.0009):
            warm_mm(1)
        with tc.tile_wait_until(0.0010):
            # S(0)A and the u0 1-column matmul share one [C,520] psum tile
            # (disjoint ranges; range-based deps keep their readers
            # independent) so they cost a single ps_s pool slot
            s_ps0a = ps_s.tile([C, 520], F32, tag="s", name="s0a")
            nc.tensor.matmul(s_ps0a[:, 0:512], y0t[:], xk_a1[:],
                             start=True, stop=True)
        with tc.tile_wait_until(0.0011):
            nc.tensor.matmul(s_ps0a[:, 512:513], xk_a1[:, 0:MT],
                             a16[:, C:C + 1], start=True, stop=True)
            shift0 = consts.tile([C, 1], F32, tag="shift0")
            nc.vector.tensor_scalar(out=shift0[:], in0=s_ps0a[:, 512:513],
                                    scalar1=-30.0, scalar2=None, op0=ALU.add)
        with tc.tile_wait_until(0.0012):
            warm_mm(1)
        with tc.tile_wait_until(0.0013):
            zp0a = ps_r.tile([C, 512], F32, tag="r", name="zp0a")
            nc.tensor.matmul(zp0a[:], a16[:, 0:C], xk_a1[:],
                             start=True, stop=True)
            nc.scalar.activation(e_stage[:, 0:512], s_ps0a[:, 0:512],
                                 AF.Exp, bias=shift0[:, 0:1])
        with tc.tile_wait_until(0.0014):
            nc.vector.tensor_scalar(out=zt[:, 0:512], in0=zp0a[:],
                                    scalar1=wf32[:, 0:1], scalar2=None,
                                    op0=ALU.add)
            # data-gated warmups: consume xk_a1 as dummy operands so they
            # become ready exactly when the real xk_a2 wait begins, keeping
            # the PE clock ungated through the last input-DMA gap
            for _ in range(2):
                nc.tensor.matmul(wps[:, 0:512], xk_a1[:, 0:C], xk_a1[:],
                                 start=True, stop=True)
        with tc.tile_wait_until(0.0015):
            s_ps0b = ps_s.tile([C, 512], F32, tag="s", name="s0b")
            nc.tensor.matmul(s_ps0b[:], y0t[:], xk_a2[:],
                             start=True, stop=True)
        with tc.tile_wait_until(0.0016):
            zp0b = ps_o.tile([C, 512], F32, tag="o", name="zp0b")
            nc.tensor.matmul(zp0b[:], a16[:, 0:C], xk_a2[:],
                             start=True, stop=True)
            nc.scalar.activation(e_stage[:, 512:PW], s_ps0b[:],
                                 AF.Exp, bias=shift0[:, 0:1])
        with tc.tile_wait_until(0.0017):
            nc.vector.tensor_scalar(out=zt[:, 512:PW], in0=zp0b[:],
                                    scalar1=wf32[:, 0:1], scalar2=None,
                                    op0=ALU.add)

        acc = [None] * N_P
        o_t = [None] * N_P      # pass 0: (o_psA, o_psB); pass 1: [C,1024]
        r_t = [None] * N_P      # pass 0: [C,1024];       pass 1: (rA, rB)

        def do_o(gg):
            p, mt = divmod(gg, N_MT)
            st = (mt == 0)
            sp = (mt == N_MT - 1)
            if p == 0:
                if st:
                    o_t[0] = (ps_o.tile([C, 512], F32, tag="o", name="o0a"),
                              ps_o.tile([C, 512], F32, tag="o", name="o0b"))
                for j in range(2):
                    nc.tensor.matmul(
                        o_t[0][j][:], xt_tile(mt),
                        e_stage[:, mt * PW + j * 512:mt * PW + (j + 1) * 512],
                        start=st, stop=sp)
            else:
                if st:
                    o_t[1] = ps_r.tile([C, PW], F32, tag="r", name="o1")
                mm(o_t[1][:], xt_tile(mt),
                   e_stage[:, mt * PW:(mt + 1) * PW], start=st, stop=sp)

        def rstage(p, stage):
            # 3-stage row-sum: stage 0 reads acc (complete through m-tile
            # 29), stages 1-2 read the last two E-tiles straight from the
            # stage buffer so the reciprocal can start right after the
            # final exp of the pass
            st = (stage == 0)
            sp = (stage == 2)
            if stage == 0:
                rhs = acc[p][:]
            else:
                emt = N_MT - 3 + stage  # 30, 31
                rhs = e_stage[:, emt * PW:(emt + 1) * PW]
            if p == 0:
                if st:
                    r_t[0] = ps_r.tile([C, PW], F32, tag="r", name="r0")
                mm(r_t[0][:], ones_bf[:], rhs, start=st, stop=sp)
            else:
                # pass-1 rowsums in the (long-free) ps_o slots; separate
                # half-tiles so each reciprocal gates on its own stage-2
                if st:
                    r_t[1] = (ps_o.tile([C, 512], F32, tag="o", name="r1a"),
                              ps_o.tile([C, 512], F32, tag="o", name="r1b"))
                for j in range(2):
                    nc.tensor.matmul(r_t[1][j][:], ones_bf[:],
                                     rhs[:, j * 512:(j + 1) * 512],
                                     start=st, stop=sp)

        def tail(p):
            # bc = 1/rowsum; out = relu(O*bc + x); pass-0 all on DVE
            # (non-critical, runs under pass-1's stream); pass-1 interleaved
            # per half across DVE/GpSimd/ACT to shorten the serial chain.
            # t2/t3 in bf16 so the residual-add and relu run in the DVE 2x
            # perf mode (the O*bc mult reads fp32 PSUM, stuck at 1x)
            bc = sb_tail.tile([C, PW], F32, tag="bc")
            t2 = sb_tail.tile([C, PW], BF16, tag="t2")
            t3 = sb_tail.tile([C, PW], BF16, tag="t3")
            oo = sb_tail.tile([C, PW], F16, tag="oo")
            if p == 0:
                nc.vector.reciprocal_approx_fast(bc[:], r_t[0][:])
                for j in range(2):
                    sl = slice(j * 512, (j + 1) * 512)
                    nc.vector.tensor_tensor(t2[:, sl], o_t[0][j][:],
                                            bc[:, sl], op=ALU.mult)
                    nc.vector.tensor_tensor(t3[:, sl], t2[:, sl],
                                            xq16[:, j * 512:(j + 1) * 512],
                                            op=ALU.add)
                    nc.vector.tensor_scalar_max(oo[:, sl], t3[:, sl], 0.0)
                    nc.sync.dma_start(out_ext[:, j * 512:(j + 1) * 512],
                                      oo[:, sl])
                return
            # pass-1 critical tail.  ACT (idle after the last exp)
            # evacuates the O psum halves to SBUF bf16, so every DVE op
            # runs in the 2x perf mode; reciprocals are split per half so
            # each gates only on its own rowsum stage-2.  Half-A's output
            # DMA issues mid-chain.
            o16 = sb_tail.tile([C, PW], BF16, tag="o16")
            nc.vector.reciprocal_approx_fast(bc[:, 0:512], r_t[1][0][:])
            nc.vector.reciprocal_approx_fast(bc[:, 512:PW], r_t[1][1][:])
            for j in range(2):
                sl = slice(j * 512, (j + 1) * 512)
                nc.scalar.activation(o16[:, sl], o_t[1][:, sl], AF.Identity,
                                     bias=zero0[:, 0:1])
                nc.vector.tensor_tensor(t2[:, sl], o16[:, sl], bc[:, sl],
                                        op=ALU.mult)
                nc.vector.tensor_tensor(t3[:, sl], t2[:, sl],
                                        xq16[:, PW + j * 512:PW + (j + 1) * 512],
                                        op=ALU.add)
                nc.vector.tensor_scalar_max(oo[:, sl], t3[:, sl], 0.0)
                nc.sync.dma_start(out_ext[:, PW + j * 512:PW + (j + 1) * 512],
                                  oo[:, sl])

        for g in range(1, TOT + DLAG):
            # per-iteration scheduler floor: pins every engine's queue to
            # the emission order of the pipeline (the scheduler's sim would
            # otherwise hoist late-data ops ahead of earlier ones)
            gctx = tc.tile_wait_until(0.003 + (g - 1) * 0.0012)
            gctx.__enter__()
            p, mt = divmod(g, N_MT)
            if g < TOT:
                s_ps = ps_s.tile([C, PW], F32, tag="s")
                mm(s_ps[:], xk_tile(mt), zt[:, p * PW:(p + 1) * PW])
                e_g = e_stage[:, mt * PW:(mt + 1) * PW]
                nc.scalar.activation(e_g, s_ps[:], AF.Exp,
                                     bias=shift[:, 0:1])
            # row-sum stages for pass 0 (mid-stream, original spots);
            # pass-1's are interleaved with the drain below
            if g == N_MT - 1:
                rstage(0, 0)
            elif g == N_MT:
                rstage(0, 1)
            elif g == N_MT + 1:
                rstage(0, 2)
            elif g == TOT - 2:
                rstage(1, 0)
            if g in (6, 8):
                # Z1 halves late enough that the xk_b bulk DMA has landed
                # by the time the in-order PE reaches them
                jj = (g - 6) // 2
                zp1 = ps_r.tile([C, 512], F32, tag="r", name=f"zp1{jj}")
                nc.tensor.matmul(zp1[:], a16[:, 0:C],
                                 xk_b[:, jj * 512:(jj + 1) * 512],
                                 start=True, stop=True)
                nc.vector.tensor_scalar(
                    out=zt[:, PW + jj * 512:PW + (jj + 1) * 512],
                    in0=zp1[:], scalar1=wf32[:, 0:1], scalar2=None,
                    op0=ALU.add)
            if g == 12:
                nc.vector.tensor_copy(xq16[:, 0:512], xk_a1[:])
                nc.vector.tensor_copy(xq16[:, 512:PW], xk_a2[:])
            if g == 44:
                nc.vector.tensor_copy(xq16[:, PW:NQ], xk_b[:, 0:PW])
            if g < TOT:
                p, mt = divmod(g, N_MT)
                # DVE row-sum accumulation (m-tiles 0..29; last two are
                # picked up directly by rstages 1-2)
                if mt == 1:
                    acc[p] = sb_acc.tile([C, PW], BF16, tag="acc", name=f"acc{p}")
                    nc.vector.tensor_tensor(acc[p][:], e_stage[:, 0:PW],
                                            e_g, op=ALU.add)
                elif 2 <= mt <= N_MT - 3:
                    nc.vector.tensor_tensor(acc[p][:], acc[p][:], e_g,
                                            op=ALU.add)
            # O-matmul emission: lag-3 mid-stream; the last five O's and the
            # pass-1 rowsum stages are spread so the PE backlog after the
            # final exp is just [O62, r-stage2, O63]
            if g >= DLAG and g - DLAG < TOT - 5:
                do_o(g - DLAG)
            elif g == TOT - 2:
                do_o(TOT - 5)
            elif g == TOT - 1:
                do_o(TOT - 4)
                do_o(TOT - 3)
                rstage(1, 1)
            elif g == TOT:
                do_o(TOT - 2)
                rstage(1, 2)
            elif g == TOT + 1:
                do_o(TOT - 1)
            for pp in range(N_P):
                # after do_o(pp, 31): the O accumulator is complete
                if g == pp * N_MT + N_MT - 1 + DLAG:
                    tail(pp)
            gctx.__exit__(None, None, None)

    nc.compile()
    return nc


_NC_CACHE = None


def _get_nc():
    global _NC_CACHE
    if _NC_CACHE is None:
        _NC_CACHE = build_nc()
    return _NC_CACHE


def make_in_maps(x, W1, b1, W2, b2):
    x = np.asarray(x, np.float32)
    W1 = np.asarray(W1, np.float32)
    b1 = np.asarray(b1, np.float32)
    W2 = np.asarray(W2, np.float32)
    b2 = np.asarray(b2, np.float32)
    A = (W1.T @ W2).astype(np.float16)          # lhsT of the Z-projection
    w = W2.T @ b1                               # folded u-bias
    A16W = np.zeros((C, 2 * C + 2), np.float16)
    A16W[:, :C] = A
    A16W[:, C] = w.astype(np.float16)
    A16W[:, C + 2:2 * C + 2] = (W2.T @ W1).astype(np.float16)
    in_maps = []
    for core in range(8):
        b, h = divmod(core, 2)
        xb = x[b]                               # [128, 4096]
        # rotate keys so this core's query half is columns 0:2048
        xrot = np.concatenate([xb[:, h * NQ:], xb[:, :h * NQ]], axis=1)
        xk16 = xrot.astype(np.float16)
        # xt[m, mt*128 + c] = xrot[c, mt*128 + m]
        xtt = np.ascontiguousarray(
            xrot.T.reshape(N_MT, MT, C).transpose(1, 0, 2).reshape(MT, N_MT * C)
        ).astype(ml_dtypes.bfloat16)
        in_maps.append({"a16": A16W, "xk": xk16, "xt": xtt})
    return in_maps


def run(x, W1, b1, W2, b2, trace=False):
    nc = _get_nc()
    in_maps = make_in_maps(x, W1, b1, W2, b2)
    last_err = None
    for _attempt in range(3):
        try:
            res = run_bass_kernel_spmd(nc, in_maps, core_ids=list(range(8)),
                                       trace=trace)
            break
        except Exception as e:  # transient NRT/device errors: retry
            last_err = e
    else:
        raise last_err
    out = np.empty((B, C, N), np.float32)
    for core in range(8):
        b, h = divmod(core, 2)
        out[b][:, h * NQ:(h + 1) * NQ] = \
            res.results[core]["out"].astype(np.float32)
    return out, res


def kernel(x, W1, b1, W2, b2):
    out, _ = run(x, W1, b1, W2, b2, trace=False)
    return out

